# revision 21
# baseline (speedup 1.0000x reference)
"""Trainium2 Bass kernel for the composed hinged (discriminative) loss.

Shapes (hardcoded): out [4,32,512,512] f32, target [4,512,512] i32,
centers [4,16,2] i32, K=16.

Sharding: data-parallel, 2 cores per image (split along H into halves),
8 cores total. Each core computes, for its 131072 pixels, the masked
hinged-distance partial sums against all 16 centers of its image.
Everything else (center embeddings, repel/reg terms, counts, the B-scan)
is O(K) or O(HW) host work.

Device layout per core (P=131072 pixels, superchunks of SC=8192):
  pixel px(q, j, i) = xoff + 2048q + 512j + i, with j = 2c + h
  (c = psum col-block 0..1, h = psum partition half, q = quadrant).
  psum[64h+16q+k, 512c+i] = d2~ of that pixel vs center k:
    m2 (bf16, Kc=16): x2 hi+lo rows via block-one lhsT  (start=True)
    m1 (bf16, Kc=128): block-diag -2E lhsT, 4 pixel groups per pass
  Then ACT sqrt(psum + e2[k]+eps) -> ACT relu(.-0.1) -> DVE is_equal
  (labels vs per-partition scalar) -> DVE scalar_tensor_tensor
  (hinged*mask, fused add-reduce) -> acc[:, sc].

Numerics: x is cast to bf16 on the host and E is gathered from the
bf16 image; bf16*bf16 products are exact in f32, and x2 is computed
from the bf16 x then shipped as a bf16 hi+lo pair, so d2~ at any
(center, k) pair is an exactly-computed ||E_m - E_k||^2 up to ~1e-5.
EPS=1e-3 then guarantees sqrt sees no negative input (no NaN), while
biasing every distance by <1e-3/16 (negligible).
"""

import os
import sys

import numpy as np

for _p in ("/opt/trn_rl_repo",):
    if _p not in sys.path and os.path.isdir(_p):
        sys.path.insert(0, _p)

import ml_dtypes  # noqa: E402

import concourse.bass as bass  # noqa: E402
import concourse.bacc as bacc  # noqa: E402
import concourse.tile as tile  # noqa: E402
from concourse import mybir  # noqa: E402
from concourse.bass_utils import run_bass_kernel_spmd  # noqa: E402

F32 = mybir.dt.float32
BF16 = mybir.dt.bfloat16
U8 = mybir.dt.uint8
BF = ml_dtypes.bfloat16

DELTA_A = np.float32(0.1)
DELTA_R = np.float32(1.0)
ALPHA, BETA, GAMMA = 1.0, 1.0, 0.001
EPS = np.float32(1e-3)
K = 16
D = 32

P_CORE = 131072  # pixels per core (half of a 512x512 image)
SC = 8192  # pixels per superchunk
NSC = P_CORE // SC
N_CORES = 8

TRACE = bool(os.environ.get("CHL_TRACE"))
last_results = None


def _dap(handle, offset, dims):
    """Custom strided AP over a DRAM tensor (element offsets)."""
    a = handle[tuple(slice(None) for _ in handle.shape)]
    return bass.AP(tensor=a.tensor, offset=offset, ap=[list(d) for d in dims])


def _build_program(p_core=P_CORE, sc=SC):
    nsc = p_core // sc
    assert sc == 8192
    nc = bacc.Bacc(None, target_bir_lowering=False)

    x_d = nc.dram_tensor("xin", [D, p_core], BF16, kind="ExternalInput")
    v2_d = nc.dram_tensor("v2in", [16 * nsc, 1024], BF16, kind="ExternalInput")
    t_d = nc.dram_tensor("tin", [nsc, 128, 1024], U8, kind="ExternalInput")
    eb4_d = nc.dram_tensor("eb4", [128, 64], BF16, kind="ExternalInput")
    ones16_d = nc.dram_tensor("ones16", [16, 128], BF16, kind="ExternalInput")
    biasv_d = nc.dram_tensor("biasv", [128, 1], F32, kind="ExternalInput")
    labv_d = nc.dram_tensor("labv", [128, 1], F32, kind="ExternalInput")
    acc_d = nc.dram_tensor("acc", [128, nsc], F32, kind="ExternalOutput")

    with tile.TileContext(nc) as tc:
        with (
            tc.tile_pool(name="singles", bufs=1) as singles,
            tc.tile_pool(name="loads", bufs=4) as loads,
            tc.tile_pool(name="work", bufs=3) as work,
            tc.tile_pool(name="ps", bufs=3, space="PSUM") as pspool,
            tc.tile_pool(name="warm", bufs=1, space="PSUM") as warmpool,
        ):
            eb4_sb = singles.tile([128, 64], BF16)
            nc.sync.dma_start(eb4_sb[:, :], eb4_d[:, :])
            ones16_sb = singles.tile([16, 128], BF16)
            nc.sync.dma_start(ones16_sb[:, :], ones16_d[:, :])
            biasv_sb = singles.tile([128, 1], F32)
            nc.sync.dma_start(biasv_sb[:, :], biasv_d[:, :])
            labv_sb = singles.tile([128, 1], F32)
            nc.sync.dma_start(labv_sb[:, :], labv_d[:, :])
            negd_sb = singles.tile([128, 1], F32)
            nc.vector.memset(negd_sb[:, :], -float(DELTA_A))
            acc_sb = singles.tile([128, nsc], F32)

            # HAM warmup: ~24 back-to-back matmuls trip the PE clock gate
            # to 8/8 (2.4 GHz) while the first loads are still in flight.
            wsrc = singles.tile([128, 512], BF16)
            nc.vector.memset(wsrc[:, :], 0.0)
            wps = warmpool.tile([128, 512], F32)
            for _ in range(24):
                nc.tensor.matmul(
                    wps[:, :],
                    lhsT=wsrc[:, 0:128],
                    rhs=wsrc[:, :],
                    start=True,
                    stop=True,
                    skip_group_check=True,
                )

            for isc in range(nsc):
                xoff = isc * sc
                fd = sc // 8  # 1024

                # x packed [128, 2048] bf16: partition 32q+d,
                # col 512j+i <-> pixel xoff + 2048q + col (contiguous!)
                x4 = loads.tile([128, sc // 4], BF16)
                nc.gpsimd.dma_start(
                    x4[:, :],
                    _dap(x_d, xoff, [[2048, 4], [p_core, D], [1, 2048]]),
                )

                # x2 hi/lo rows for this superchunk: [u, 512c+i]
                v2t = loads.tile([16, fd], BF16)
                nc.sync.dma_start(
                    v2t[:, :], v2_d[16 * isc : 16 * isc + 16, :]
                )
                # labels replicated: [64h+16q+k, 512c+i]
                trept = loads.tile([128, fd], U8)
                nc.sync.dma_start(trept[:, :], t_d[isc, :, :])

                ps = pspool.tile([128, fd], F32)
                for c in range(2):
                    nc.tensor.matmul(
                        ps[:, 512 * c : 512 * c + 512],
                        lhsT=ones16_sb[:, :],
                        rhs=v2t[:, 512 * c : 512 * c + 512],
                        start=True,
                        stop=False,
                        skip_group_check=True,
                    )
                for c in range(2):
                    for h in range(2):
                        j = 2 * c + h
                        nc.tensor.matmul(
                            ps[64 * h : 64 * h + 64, 512 * c : 512 * c + 512],
                            lhsT=eb4_sb[:, :],
                            rhs=x4[:, 512 * j : 512 * j + 512],
                            start=False,
                            stop=(h == 1),
                            skip_group_check=True,
                        )

                bsb = work.tile([128, fd], F32)
                nc.scalar.activation(
                    bsb[:, :],
                    ps[:, :],
                    mybir.ActivationFunctionType.Sqrt,
                    bias=biasv_sb[:, 0:1],
                    scale=1.0,
                )
                hsb = work.tile([128, fd], F32)
                nc.scalar.activation(
                    hsb[:, :],
                    bsb[:, :],
                    mybir.ActivationFunctionType.Relu,
                    bias=negd_sb[:, 0:1],
                )
                msk = work.tile([128, fd], F32)
                nc.vector.tensor_scalar(
                    msk[:, :],
                    trept[:, :],
                    labv_sb[:, 0:1],
                    None,
                    mybir.AluOpType.is_equal,
                )
                scr = work.tile([128, fd], F32)
                nc.vector.scalar_tensor_tensor(
                    scr[:, :],
                    hsb[:, :],
                    0.0,
                    msk[:, :],
                    mybir.AluOpType.add,
                    mybir.AluOpType.mult,
                    accum_out=acc_sb[:, isc : isc + 1],
                )

            nc.sync.dma_start(acc_d[:, :], acc_sb[:, :])

    nc.finalize()
    return nc


_program_cache = {}


def _get_program(p_core=P_CORE, sc=SC):
    key = (p_core, sc)
    if key not in _program_cache:
        _program_cache[key] = _build_program(p_core, sc)
    return _program_cache[key]


def _rep_reg_jax(E):
    """s_rep, s_reg computed exactly as the jax reference does (CPU f32)."""
    import jax
    import jax.numpy as jnp

    with jax.default_device(jax.devices("cpu")[0]):
        Ek = jnp.asarray(E.T)  # [K, D], matches reference's E

        def safe_sqrt(x):
            pos = x > 0
            return jnp.where(pos, jnp.sqrt(jnp.where(pos, x, 1.0)), 0.0)

        d2 = (
            jnp.sum(Ek * Ek, 1)[:, None]
            + jnp.sum(Ek * Ek, 1)[None, :]
            - 2.0 * Ek @ Ek.T
        )
        nE = safe_sqrt(jax.nn.relu(d2))
        s_rep = jnp.sum(jax.nn.relu(DELTA_R - nE)) - K * DELTA_R
        s_reg = jnp.sum(safe_sqrt(jnp.sum(Ek * Ek, axis=1)))
        return float(s_rep), float(s_reg)


def _arrange_v2(x2_half):
    """x2 (f32, [131072]) -> v2_all [16*NSC, 1024] bf16 hi/lo rows.

    Row 16*sc + (lohi*8 + 4h+q), col 512c+i."""
    hi = x2_half.astype(BF)
    lo = (x2_half - hi.astype(np.float32)).astype(BF)
    out = np.empty((16 * NSC, 1024), BF)
    for arr, off in ((hi, 0), (lo, 8)):
        v = arr.reshape(NSC, 4, 2, 2, 512)  # (sc, q, c, h, i)
        t = v.transpose(0, 3, 1, 2, 4).reshape(NSC, 8, 1024)  # (sc, (h,q), (c,i))
        for u in range(8):
            out[16 * np.arange(NSC) + off + u, :] = t[:, u, :]
    return out


def _arrange_trep(t8_half):
    """t8 (u8, [131072]) -> t_rep_all [128, 1024*NSC] u8."""
    v = t8_half.reshape(NSC, 4, 2, 2, 512)  # (sc, q, c, h, i)
    t = v.transpose(3, 1, 0, 2, 4)  # (h, q, sc, c, i)
    t = np.broadcast_to(t[:, :, None], (2, 4, K, NSC, 2, 512))
    arr = np.ascontiguousarray(t.reshape(128, NSC, 1024))
    return np.ascontiguousarray(arr.transpose(1, 0, 2))


def _host_prep(out, target, centers):
    B = out.shape[0]
    per_image = []
    in_maps = []
    for b in range(B):
        r = centers[b, :, 0].astype(np.int64)
        c = centers[b, :, 1].astype(np.int64)
        E = out[b][:, r, c].astype(np.float32)  # [D, K] full precision
        xbf = out[b].astype(BF)  # [D, 512, 512]
        Ebf32 = xbf[:, r, c].astype(np.float32)  # [D, K] bf16-rounded
        e2 = np.sum(Ebf32 * Ebf32, axis=0, dtype=np.float32)  # [K]
        lab_raw = target[b][r, c].astype(np.int64)
        uniq = np.unique(lab_raw)
        lab_id = np.searchsorted(uniq, lab_raw).astype(np.int64)
        tb = target[b].reshape(-1)
        t8 = np.full(tb.shape, 255, np.uint8)
        for j, v in enumerate(uniq):
            t8[tb == v] = j
        hist = np.bincount(t8, minlength=256)
        cnt = hist[lab_id]
        denom = np.maximum(cnt - 1, 1).astype(np.float32)

        eb4 = np.zeros((128, 64), BF)
        for q in range(4):
            eb4[32 * q : 32 * q + 32, 16 * q : 16 * q + 16] = (
                -2.0 * Ebf32
            ).astype(BF)
        ones16 = np.zeros((16, 128), BF)
        for h in range(2):
            for q in range(4):
                u = 4 * h + q
                sl = np.s_[64 * h + 16 * q : 64 * h + 16 * q + 16]
                ones16[u, sl] = 1.0
                ones16[u + 8, sl] = 1.0
        biasv = np.tile(e2 + EPS, 8).reshape(128, 1).astype(np.float32)
        labv = np.tile(lab_id.astype(np.float32), 8).reshape(128, 1)

        x2b = np.sum(
            xbf.astype(np.float32) ** 2, axis=0, dtype=np.float32
        ).reshape(512, 512)

        per_image.append(dict(E=E, denom=denom))
        t8img = t8.reshape(512, 512)
        for half in range(2):
            rows = slice(256 * half, 256 * (half + 1))
            in_maps.append(
                {
                    "xin": np.ascontiguousarray(xbf[:, rows, :].reshape(D, -1)),
                    "v2in": _arrange_v2(
                        np.ascontiguousarray(x2b[rows, :].reshape(-1))
                    ),
                    "tin": _arrange_trep(
                        np.ascontiguousarray(t8img[rows, :].reshape(-1))
                    ),
                    "eb4": eb4,
                    "ones16": ones16,
                    "biasv": biasv,
                    "labv": labv,
                }
            )
    return per_image, in_maps


def kernel(out, target, centers, batch_size=None, **_unused):
    global last_results
    out = np.asarray(out, dtype=np.float32)
    target = np.asarray(target, dtype=np.int32)
    centers = np.asarray(centers, dtype=np.int32)
    B = out.shape[0]

    per_image, in_maps = _host_prep(out, target, centers)

    nc = _get_program()
    res = run_bass_kernel_spmd(
        nc, in_maps, core_ids=list(range(N_CORES)), trace=TRACE
    )
    last_results = res

    s_att = np.zeros(B, np.float64)
    s_rep = np.zeros(B, np.float64)
    s_reg = np.zeros(B, np.float64)
    for b in range(B):
        hing = np.zeros(K, np.float64)
        for half in range(2):
            acc = np.asarray(res.results[2 * b + half]["acc"], np.float64)
            hing += acc.reshape(8, K, -1).sum(axis=(0, 2))
        info = per_image[b]
        s_att[b] = float(np.sum(hing / info["denom"].astype(np.float64)))
        sr, sg = _rep_reg_jax(info["E"])
        s_rep[b] = sr
        s_reg[b] = sg

    div_att = np.float32(K)
    div_rep = np.float32(K * (K - 1))
    div_reg = np.float32(K)
    a = np.float32(0.0)
    r_ = np.float32(0.0)
    g = np.float32(0.0)
    for b in range(B):
        a = np.float32((a + np.float32(s_att[b])) / div_att)
        r_ = np.float32((r_ + np.float32(s_rep[b])) / div_rep)
        g = np.float32((g + np.float32(s_reg[b])) / div_reg)
    loss = np.float32(ALPHA * a + BETA * r_ + GAMMA * g)
    return loss, a, r_
